# revision 8
# baseline (speedup 1.0000x reference)
"""Causal self-attention on 8 Trainium2 NeuronCores.

Tensor-parallel by heads: each core owns 2 of the 16 heads end-to-end
(QKV projection -> causal attention -> row-sharded output projection),
and the 8 partial projection outputs are summed on the host.

v2 schedule (vs the first working version):
  - exp is split between the Scalar ACT engine (spline Exp) and a custom
    DVE op EXP_SQ16_ANT (exp(s*x) = ((s'/2 x + s') x + 1)^16, s' = s/16;
    8 ALU stages, rel err <6e-3 over the observed score range) so the
    attention phase is no longer ACT-bound.
  - softmax denominators: the attnV ones-column rows are copied to a
    2-row stage, reciprocal'd there, and partition-broadcast on GpSimd
    (no DRAM bounce round-trip).
  - psum evacuations are spread across DVE / ACT / GpSimd; q-bias and
    k-bias adds keep their scalar operand pre-copied on the same engine
    (TensorScalarPtr has a single wait slot).
  - all 8 x m-tiles are prefetched up front across four engine DMA
    queues; output DMAs ride the sync queue (no engine issue cost).
  - proj(b0) is emitted between b1's first and last QKV m-tiles,
    proj(b1, qt0-2) before the last qt's normalize completes, so the PE
    never idles at phase boundaries.  proj psum rides the "ps" ring
    (idle outside attention), pyA/pyB the 4-slot "py" ring.
"""

import os
import numpy as np
from contextlib import ExitStack

import concourse.bass as bass
import concourse.mybir as mybir
import concourse.tile as tile
from concourse import bacc

B, T, C, H, D = 2, 2048, 1024, 16, 64
NCORES = 8
HPC = H // NCORES          # heads per core = 2
BT = B * T                 # 4096 tokens
P = 128
KO = C // P                # 8 contraction chunks of 128
MT = 512                   # qkv m-tile (tokens)
NMT_B = T // MT            # 4 m-tiles per batch
QTW = 512                  # q tile width
NQT = T // QTW             # 4
NKB = T // P               # 16 k-blocks per batch
SCALE = 1.0 / np.sqrt(D)   # 0.125
F32 = mybir.dt.float32
BF16 = mybir.dt.bfloat16
MMDT = BF16

LAST_RESULT = None  # BassKernelResults of the most recent run (for profiling)

# ---------------------------------------------------------------------------
# Custom DVE op: exp(SCALE*x) ~= ((x*c0 + c1)*x + 1)^16 with c1 = SCALE/16,
# c0 = c1^2/2.  Exactly 8 ALU stages (mult, add, mult, add, 4x square).
# Registered into concourse.dve_ops' tables at import (additive only).
# ---------------------------------------------------------------------------
_EXP_C1 = float(SCALE / 16.0)
_EXP_C0 = float(_EXP_C1 * _EXP_C1 / 2.0)


def _exp_sq16_ref(in0, in1, s0, s1, imm2):
    x = in0.astype(np.float32)
    q = (x * np.float32(s0) + np.float32(s1)) * x + np.float32(1.0)
    for _ in range(4):
        q = (q * q).astype(np.float32)
    return q


def _register_exp_op():
    import concourse.dve_ops as dve_ops
    import concourse.dve_spec as dve_spec
    from concourse.dve_spec import Spec, Src0, C0, C1, One, sq
    from concourse.dve_uop import DveOpSpec

    name = "EXP_SQ16_ANT"
    for op in dve_ops.OPS:
        if op.name == name:
            return op
    spec = Spec(
        body=sq(sq(sq(sq((Src0 * C0 + C1) * Src0 + One)))),
        reference=_exp_sq16_ref,
    )
    row = dve_ops._CUSTOM_DVE_ROW_BASE + len(dve_ops.OPS)
    assert row < 0x20, "no free custom-DVE opcode row"
    shas = {}
    for ver in ("v3", "v4"):
        try:
            uops = dve_spec.lower(spec, ver=ver)
            shas[ver] = DveOpSpec(
                name=name, opcode=row, uops=uops, rd1_en=False
            ).sha(ver)
        except Exception:
            pass
    op = dve_ops.DveOp(name, spec, subdim=False, uops_sha=shas)
    dve_ops.OPS.append(op)
    dve_ops.CUSTOM_DVE_SPECS[name] = spec
    dve_ops._SUB_OPCODE_FOR_NAME[name] = row
    return op


EXP_OP = _register_exp_op()

# which groups' exp runs on the DVE custom op (the rest on ACT).  2-of-5
# keeps DVE below the PE's per-group budget alongside its normalize work.
_DVE_PAT = os.environ.get("KERNEL_DVE_EXP", "13")


def build_nc():
    nc = bacc.Bacc(target_bir_lowering=False)

    xT_d = nc.dram_tensor("xT", [C, BT], MMDT, kind="ExternalInput")
    w_d = nc.dram_tensor("w", [C, 3 * HPC * D], MMDT, kind="ExternalInput")
    bqk_d = nc.dram_tensor("bqk", [P, 2], F32, kind="ExternalInput")
    wp_d = nc.dram_tensor("wp", [P, C], MMDT, kind="ExternalInput")
    id_d = nc.dram_tensor("ident", [P, P], F32, kind="ExternalInput")
    out_d = nc.dram_tensor("out", [BT, C], BF16, kind="ExternalOutput")

    xT3 = xT_d.ap().rearrange("(ko p) m -> p ko m", p=P)   # [128, 8, 4096]
    w3 = w_d.ap().rearrange("(ko p) n -> p ko n", p=P)     # [128, 8, 384]
    out2 = out_d.ap()                                       # [4096, 1024]

    Exp = mybir.ActivationFunctionType.Exp
    Copy = mybir.ActivationFunctionType.Copy
    mult = mybir.AluOpType.mult
    add = mybir.AluOpType.add

    with tile.TileContext(nc) as tc, ExitStack() as ctx:
        const = ctx.enter_context(tc.tile_pool(name="const", bufs=1))
        big = ctx.enter_context(tc.tile_pool(name="big", bufs=1))
        xpool = ctx.enter_context(tc.tile_pool(name="xpool", bufs=8))
        epool = ctx.enter_context(tc.tile_pool(name="epool", bufs=6))
        spool = ctx.enter_context(tc.tile_pool(name="spool", bufs=2))
        opool = ctx.enter_context(tc.tile_pool(name="opool", bufs=8))
        ypool = ctx.enter_context(tc.tile_pool(name="ypool", bufs=8))
        dpool = ctx.enter_context(tc.tile_pool(name="dpool", bufs=2, space="DRAM"))
        psum = ctx.enter_context(tc.tile_pool(name="psum", bufs=1, space="PSUM"))

        # ---------------- input DMAs ----------------
        # sync queue: x g0 per-k-chunk (interleaved with w on the scalar
        # queue, so the k-th QKV matmul starts when its own slices land).
        # vector queue: wp/ident/bqk consts.  remaining x m-tiles are
        # prefetched whole on rotating engine queues.
        w_sb = const.tile([P, KO, 3 * HPC * D], MMDT)
        wp_sb = const.tile([P, C], MMDT)
        id_sb = const.tile([P, P], F32)
        bqk_sb0 = const.tile([P, 2], F32)
        nc.gpsimd.dma_start(out=wp_sb[:, :], in_=wp_d.ap()[:, :])
        nc.gpsimd.dma_start(out=id_sb[:], in_=id_d.ap())
        nc.gpsimd.dma_start(out=bqk_sb0[:], in_=bqk_d.ap())

        xms = []
        for g in range(B * NMT_B):
            xms.append(xpool.tile([P, KO, MT], MMDT, tag="xm", name=f"xm{g}"))
        for k in range(KO):
            nc.scalar.dma_start(out=w_sb[:, k, :], in_=w3[:, k, :])
            nc.sync.dma_start(out=xms[0][:, k, :], in_=xT3[:, k, 0:MT])
        qs = [nc.sync, nc.gpsimd, nc.scalar]
        for g in range(1, B * NMT_B):
            qs[g % 3].dma_start(out=xms[g][:, :, :],
                                in_=xT3[:, :, g * MT:(g + 1) * MT])

        # Pre-consume DMA semaphores on the engines that will read these
        # tiles (single-wait-slot encodings can't wait (engine, DMA)).
        bqk_q = const.tile([P, 2], F32)     # read by vector (q/k bias)
        nc.vector.tensor_copy(out=bqk_q[:], in_=bqk_sb0[:])
        idb_sb = const.tile([P, P], MMDT)   # read by PE transposes
        nc.scalar.copy(out=idb_sb[:], in_=id_sb[:])

        QT_sb = big.tile([P, B, T], MMDT)   # rows: [qA feats | qB feats]
        KT_sb = big.tile([P, B, T], MMDT)
        VT_sb = big.tile([P, B, T], MMDT)
        # merged V tile: cols 0:64 vA, 64 onesA | 65.. : VB block
        # (65+32 onesB, 65+64..65+128 vB)
        VAB = big.tile([P, B, NKB, 65 + P], MMDT)
        nc.gpsimd.memset(VAB[:], 0.0)
        nc.vector.memset(VAB[:, :, :, 64:65], 1.0)
        nc.vector.memset(VAB[:, :, :, 65 + 32:65 + 33], 1.0)

        # PE warm-up + pre-consume of wp/id DMA sems (results unused).
        pid = psum.tile([P, P], F32, tag="py", bufs=4)
        nc.tensor.transpose(pid[:], id_sb[:], id_sb[:])
        pwp = psum.tile([P, QTW], F32, tag="py", bufs=4)
        nc.tensor.matmul(pwp[:, 0:P], wp_sb[:, 0:P], wp_sb[:, 0:P],
                         start=True, stop=True)
        nc.tensor.matmul(pwp[:, 0:P], wp_sb[:, QTW:QTW + P],
                         wp_sb[:, QTW:QTW + P], start=True, stop=True)

        yts = {}
        pys = {}
        evac_rr = [0]

        def emit_qkv(b):
            for mt in range(NMT_B):
                g = b * NMT_B + mt
                xm = xms[g]
                for nch in range(3):
                    pq = psum.tile([P, MT], F32, tag="py", bufs=4)
                    for k in range(KO):
                        nc.tensor.matmul(
                            pq[:],
                            (w_sb[:, k, nch * P:(nch + 1) * P]),
                            (xm[:, k, :]),
                            start=(k == 0),
                            stop=(k == KO - 1),
                        )
                    dst = (QT_sb, KT_sb, VT_sb)[nch]
                    dslice = dst[:, b, mt * MT:(mt + 1) * MT]
                    if nch == 0:
                        nc.vector.tensor_scalar_add(
                            out=dslice, in0=pq[:], scalar1=bqk_q[:, 0:1])
                    elif nch == 1:
                        nc.vector.tensor_scalar_add(
                            out=dslice, in0=pq[:], scalar1=bqk_q[:, 1:2])
                    else:
                        nc.scalar.copy(out=dslice, in_=pq[:])

        def emit_vtrans(b):
            # V back-transpose to [token, feat]; single strided evac copy
            # per block into the merged VAB tile (A cols 0:64, B 65+64:).
            for kb in range(NKB):
                pt = psum.tile([P, P], MMDT, tag="py", bufs=4)
                nc.tensor.transpose(
                    pt[:], VT_sb[:, b, kb * P:(kb + 1) * P], idb_sb[:])
                dst = VAB[:, b, kb, :]
                dstv = bass.AP(tensor=dst.tensor, offset=dst.offset,
                               ap=[list(dst.ap[0]), [65 + 64, 2], [1, 64]])
                eng = (nc.vector, nc.scalar)[evac_rr[0] % 2]
                evac_rr[0] += 1
                src = pt.rearrange("p (two f) -> p two f", two=2)
                if eng is nc.scalar:
                    eng.copy(out=dstv, in_=src)
                else:
                    eng.tensor_copy(out=dstv, in_=src)

        use_pb = os.environ.get("KERNEL_PB", "1") == "1"

        def emit_normalize(b, qt):
            pyA, pyB = pys[(b, qt)]
            if not use_pb:
                # fallback: baseline-style DRAM bounce broadcast
                yu = spool.tile([P, 2 * QTW], F32, tag="yu", bufs=4,
                                name=f"yu_{b}_{qt}")
                nc.vector.tensor_copy(out=yu[0:65, 0:QTW], in_=pyA[0:65, :])
                nc.vector.tensor_copy(out=yu[0:128, QTW:2 * QTW], in_=pyB[:, :])
                dr = dpool.tile([2, QTW], F32, tag="dr", bufs=3, name=f"dr_{b}_{qt}")
                nc.sync.dma_start(out=dr[1:2, :], in_=yu[64:65, 0:QTW])
                nc.sync.dma_start(out=dr[0:1, :], in_=yu[32:33, QTW:2 * QTW])
                dbx = spool.tile([P, QTW], F32, tag="db", bufs=2, name=f"db_{b}_{qt}")
                rowB, rowA = dr[0:1, :], dr[1:2, :]
                srcA = bass.AP(tensor=rowA.tensor, offset=rowA.offset,
                               ap=[[0, 64], [1, QTW]])
                srcB = bass.AP(tensor=rowB.tensor, offset=rowB.offset,
                               ap=[[0, 64], [1, QTW]])
                nc.sync.dma_start(out=dbx[0:64, :], in_=srcA)
                nc.sync.dma_start(out=dbx[64:128, :], in_=srcB)
                rb = spool.tile([P, QTW], F32, tag="rb", bufs=3, name=f"rb_{b}_{qt}")
                nc.vector.reciprocal_approx_fast(out=rb[:, :], in_=dbx[:, :])
                yTq = ypool.tile([P, QTW], MMDT, tag="yT", name=f"yT_{b}_{qt}")
                yts[(b, qt)] = yTq
                nc.vector.tensor_tensor(
                    yTq[0:64, :], yu[0:64, 0:QTW], rb[0:64, :], mult)
                nc.vector.tensor_tensor(
                    yTq[64:128, :], yu[64:128, QTW:2 * QTW], rb[64:128, :], mult)
                return
            # denominator rows -> 2-partition stage, reciprocal there,
            # broadcast each head's row across its 64 partitions on gpsimd.
            dsA = spool.tile([1, QTW], F32, tag="dsA", bufs=2, name=f"dsA_{b}_{qt}")
            dsB = spool.tile([1, QTW], F32, tag="dsB", bufs=2, name=f"dsB_{b}_{qt}")
            nc.vector.tensor_copy(out=dsA[:, :], in_=pyA[64:65, :])
            nc.vector.tensor_copy(out=dsB[:, :], in_=pyB[32:33, :])
            rsA = spool.tile([1, QTW], F32, tag="rsA", bufs=2, name=f"rsA_{b}_{qt}")
            rsB = spool.tile([1, QTW], F32, tag="rsB", bufs=2, name=f"rsB_{b}_{qt}")
            nc.vector.reciprocal_approx_fast(out=rsA[:, :], in_=dsA[:, :])
            nc.vector.reciprocal_approx_fast(out=rsB[:, :], in_=dsB[:, :])
            db = spool.tile([P, QTW], F32, tag="db", bufs=2, name=f"db_{b}_{qt}")
            nc.gpsimd.partition_broadcast(db[0:64, :], rsA[0:1, :])
            nc.gpsimd.partition_broadcast(db[64:128, :], rsB[0:1, :])
            yTq = ypool.tile([P, QTW], MMDT, tag="yT", name=f"yT_{b}_{qt}")
            yts[(b, qt)] = yTq
            nc.vector.tensor_tensor(
                yTq[0:64, :], pyA[0:64, :], db[0:64, :], mult)
            nc.vector.tensor_tensor(
                yTq[64:128, :], pyB[64:128, :], db[64:128, :], mult)

        def emit_attnv(b, item):
            qt, kb, e, qoff, first, last = item
            if first:
                pyA = psum.tile([P, QTW], F32, tag="py", bufs=4,
                                name=f"pyA_{b}_{qt}")
                pyB = psum.tile([P, QTW], F32, tag="py", bufs=4,
                                name=f"pyB_{b}_{qt}")
                pys[(b, qt)] = (pyA, pyB)
            pyA, pyB = pys[(b, qt)]
            nc.tensor.matmul(
                pyA[0:65, qoff:QTW], (VAB[:, b, kb, 0:65]),
                (e[:, qoff:QTW]),
                start=first, stop=last, skip_group_check=True,
            )
            nc.tensor.matmul(
                pyB[:, qoff:QTW], (VAB[:, b, kb, 65:65 + P]),
                (e[:, QTW + qoff:2 * QTW]),
                start=first, stop=last, skip_group_check=True,
            )
            if last:
                emit_normalize(b, qt)

        def emit_proj(b, qts):
            # out[tok, :] partials; po rides the "ps" ring (idle when no
            # scores are in flight); evacs round-robin across engines.
            for qt in qts:
                yTq = yts[(b, qt)]
                for sm4 in range(4):
                    sm = qt * 4 + sm4
                    po = psum.tile([P, 2, QTW], F32, tag="ps", bufs=2,
                                   name=f"po_{b}_{sm}")
                    osb = opool.tile([P, C], BF16, tag="osb")
                    for nh in range(2):
                        nc.tensor.matmul(
                            po[:, nh, :],
                            (yTq[:, sm4 * P:(sm4 + 1) * P]),
                            (wp_sb[:, nh * QTW:(nh + 1) * QTW]),
                            start=True, stop=True,
                        )
                        eng = (nc.scalar, nc.vector)[evac_rr[0] % 2]
                        evac_rr[0] += 1
                        if eng is nc.scalar:
                            eng.copy(out=osb[:, nh * QTW:(nh + 1) * QTW],
                                     in_=po[:, nh, :])
                        else:
                            eng.tensor_copy(out=osb[:, nh * QTW:(nh + 1) * QTW],
                                            in_=po[:, nh, :])
                    r0 = b * T + sm * P
                    nc.sync.dma_start(out=out2[r0:r0 + P, :], in_=osb[:])

        def emit_attention(b):
            # One flat stream of score-groups, diagonal blocks first within
            # each qt; attnV trails scores by SKEW groups across qt
            # boundaries so the PE never drains waiting on exp.
            SKEW = 4
            groups = []
            for qt in range(NQT):
                order = list(range(qt * 4, (qt + 1) * 4)) + list(range(0, qt * 4))
                for i, kb in enumerate(order):
                    groups.append((qt, kb, i == 0, i == len(order) - 1))

            pend = []
            for gi, (qt, kb, first, last) in enumerate(groups):
                d = kb - (qt * (QTW // P))
                qoff = 0 if os.environ.get("KERNEL_NARROW", "1") == "0" \
                    else max(0, d) * P
                w = QTW - qoff
                q0 = qt * QTW + qoff
                ps = psum.tile([P, 2 * QTW], F32, tag="ps", bufs=2,
                               name=f"ps_{b}_{qt}_{kb}")
                nc.tensor.matmul(
                    ps[:, qoff:QTW],
                    (KT_sb[0:64, b, kb * P:(kb + 1) * P]),
                    (QT_sb[0:64, b, q0:q0 + w]),
                    start=True, stop=True, tile_position=(0, 0),
                )
                nc.tensor.matmul(
                    ps[:, QTW + qoff:2 * QTW],
                    (KT_sb[64:128, b, kb * P:(kb + 1) * P]),
                    (QT_sb[64:128, b, q0:q0 + w]),
                    start=True, stop=True, tile_position=(64, 0),
                )
                e = epool.tile([P, 2 * QTW], MMDT, tag="e", bufs=6,
                               name=f"e_{b}_{qt}_{kb}")
                ps3 = ps.rearrange("p (h q) -> p h q", h=2)
                e3 = e.rearrange("p (h q) -> p h q", h=2)
                use_dve = (str(gi % 5) in _DVE_PAT) and EXP_OP is not None
                if use_dve:
                    nc.vector._custom_dve(
                        EXP_OP, out=e3[:, :, qoff:], in0=ps3[:, :, qoff:],
                        s0=_EXP_C0, s1=_EXP_C1)
                else:
                    nc.scalar.activation(out=e3[:, :, qoff:],
                                         in_=ps3[:, :, qoff:],
                                         func=Exp, scale=SCALE)
                if d >= 0:
                    # within the window: keep exp[j, h, i'] where i' >= j
                    nc.gpsimd.affine_select(
                        out=e3[:, :, qoff:],
                        in_=e3[:, :, qoff:],
                        pattern=[[0, 2], [1, w]],
                        compare_op=mybir.AluOpType.is_ge,
                        fill=0.0,
                        base=0,
                        channel_multiplier=-1,
                    )
                pend.append((qt, kb, e, qoff, first, last))
                if len(pend) > SKEW:
                    emit_attnv(b, pend.pop(0))
            for item in pend:
                emit_attnv(b, item)

        # ---------------- schedule ----------------
        emit_qkv(0)
        emit_vtrans(0)
        emit_attention(0)
        emit_qkv(1)           # PE covers b0's last normalize latency
        emit_vtrans(1)
        emit_proj(0, range(NQT))
        emit_attention(1)
        emit_proj(1, range(NQT - 1))   # qt3's normalize drains under these
        emit_proj(1, [NQT - 1])

    nc.finalize()
    return nc


def prep_inputs(x, W_qkv, b_qkv, W_proj, b_proj):
    """Host-side sharding: returns list of 8 per-core input dicts."""
    import ml_dtypes
    mmnp = ml_dtypes.bfloat16
    x = np.asarray(x, dtype=np.float32)
    W_qkv = np.asarray(W_qkv, dtype=np.float32)
    b_qkv = np.asarray(b_qkv, dtype=np.float32)
    W_proj = np.asarray(W_proj, dtype=np.float32)

    xT = np.ascontiguousarray(x.reshape(BT, C).T).astype(mmnp)   # [C, BT]
    ident = np.eye(P, dtype=np.float32)

    in_maps = []
    for c in range(NCORES):
        hA, hB = HPC * c, HPC * c + 1
        cols = []
        for part in range(3):                               # q, k, v
            for h in (hA, hB):
                cols.append(W_qkv[:, part * C + h * D: part * C + (h + 1) * D])
        w = np.ascontiguousarray(np.concatenate(cols, axis=1)).astype(mmnp)

        bq = np.concatenate([b_qkv[hA * D:(hA + 1) * D], b_qkv[hB * D:(hB + 1) * D]])
        bk = np.concatenate([b_qkv[C + hA * D: C + (hA + 1) * D],
                             b_qkv[C + hB * D: C + (hB + 1) * D]])
        bqk = np.ascontiguousarray(np.stack([bq, bk], axis=1))  # [128, 2]

        wp = np.ascontiguousarray(W_proj[c * P:(c + 1) * P, :]).astype(mmnp)

        in_maps.append({
            "xT": xT,
            "w": w,
            "bqk": bqk,
            "wp": wp,
            "ident": ident,
        })
    return in_maps


_NC_CACHE = None


def kernel(x, W_qkv, b_qkv, W_proj, b_proj):
    global _NC_CACHE, LAST_RESULT
    from concourse.bass_utils import run_bass_kernel_spmd

    if _NC_CACHE is None:
        _NC_CACHE = build_nc()
    nc = _NC_CACHE

    in_maps = prep_inputs(x, W_qkv, b_qkv, W_proj, b_proj)
    trace = os.environ.get("KERNEL_TRACE", "0") == "1"
    res = run_bass_kernel_spmd(nc, in_maps, list(range(NCORES)), trace=trace)
    LAST_RESULT = res

    acc = np.zeros((BT, C), dtype=np.float64)
    for r in res.results:
        acc += r["out"].astype(np.float64)
    # attn rows sum to 1, so the V bias contributes b_v @ W_proj to every
    # token row; add it and the proj bias here (exact, part of unshard).
    W_proj = np.asarray(W_proj, dtype=np.float32)
    b_qkv = np.asarray(b_qkv, dtype=np.float32)
    b_proj = np.asarray(b_proj, dtype=np.float32)
    acc += (b_qkv[2 * C:].astype(np.float64) @ W_proj.astype(np.float64)
            + b_proj.astype(np.float64))
    return acc.astype(np.float32).reshape(B, T, C)


# revision 9
# speedup vs baseline: 1.0057x; 1.0057x over previous
"""Causal self-attention on 8 Trainium2 NeuronCores.

Tensor-parallel by heads: each core owns 2 of the 16 heads end-to-end
(QKV projection -> causal attention -> row-sharded output projection),
and the 8 partial projection outputs are summed on the host.

v2 schedule (vs the first working version):
  - exp is split between the Scalar ACT engine (spline Exp) and a custom
    DVE op EXP_SQ16_ANT (exp(s*x) = ((s'/2 x + s') x + 1)^16, s' = s/16;
    8 ALU stages, rel err <6e-3 over the observed score range) so the
    attention phase is no longer ACT-bound.
  - softmax denominators: the attnV ones-column rows are copied to a
    2-row stage, reciprocal'd there, and partition-broadcast on GpSimd
    (no DRAM bounce round-trip).
  - psum evacuations are spread across DVE / ACT / GpSimd; q-bias and
    k-bias adds keep their scalar operand pre-copied on the same engine
    (TensorScalarPtr has a single wait slot).
  - all 8 x m-tiles are prefetched up front across four engine DMA
    queues; output DMAs ride the sync queue (no engine issue cost).
  - proj(b0) is emitted between b1's first and last QKV m-tiles,
    proj(b1, qt0-2) before the last qt's normalize completes, so the PE
    never idles at phase boundaries.  proj psum rides the "ps" ring
    (idle outside attention), pyA/pyB the 4-slot "py" ring.
"""

import os
import numpy as np
from contextlib import ExitStack

import concourse.bass as bass
import concourse.mybir as mybir
import concourse.tile as tile
from concourse import bacc

B, T, C, H, D = 2, 2048, 1024, 16, 64
NCORES = 8
HPC = H // NCORES          # heads per core = 2
BT = B * T                 # 4096 tokens
P = 128
KO = C // P                # 8 contraction chunks of 128
MT = 512                   # qkv m-tile (tokens)
NMT_B = T // MT            # 4 m-tiles per batch
QTW = 512                  # q tile width
NQT = T // QTW             # 4
NKB = T // P               # 16 k-blocks per batch
SCALE = 1.0 / np.sqrt(D)   # 0.125
F32 = mybir.dt.float32
BF16 = mybir.dt.bfloat16
MMDT = BF16

LAST_RESULT = None  # BassKernelResults of the most recent run (for profiling)

# ---------------------------------------------------------------------------
# Custom DVE op: exp(SCALE*x) ~= ((x*c0 + c1)*x + 1)^16 with c1 = SCALE/16,
# c0 = c1^2/2.  Exactly 8 ALU stages (mult, add, mult, add, 4x square).
# Registered into concourse.dve_ops' tables at import (additive only).
# ---------------------------------------------------------------------------
_EXP_C1 = float(SCALE / 16.0)
_EXP_C0 = float(_EXP_C1 * _EXP_C1 / 2.0)


def _exp_sq16_ref(in0, in1, s0, s1, imm2):
    x = in0.astype(np.float32)
    q = (x * np.float32(s0) + np.float32(s1)) * x + np.float32(1.0)
    for _ in range(4):
        q = (q * q).astype(np.float32)
    return q


def _register_exp_op():
    import concourse.dve_ops as dve_ops
    import concourse.dve_spec as dve_spec
    from concourse.dve_spec import Spec, Src0, C0, C1, One, sq
    from concourse.dve_uop import DveOpSpec

    name = "EXP_SQ16_ANT"
    for op in dve_ops.OPS:
        if op.name == name:
            return op
    spec = Spec(
        body=sq(sq(sq(sq((Src0 * C0 + C1) * Src0 + One)))),
        reference=_exp_sq16_ref,
    )
    row = dve_ops._CUSTOM_DVE_ROW_BASE + len(dve_ops.OPS)
    assert row < 0x20, "no free custom-DVE opcode row"
    shas = {}
    for ver in ("v3", "v4"):
        try:
            uops = dve_spec.lower(spec, ver=ver)
            shas[ver] = DveOpSpec(
                name=name, opcode=row, uops=uops, rd1_en=False
            ).sha(ver)
        except Exception:
            pass
    op = dve_ops.DveOp(name, spec, subdim=False, uops_sha=shas)
    dve_ops.OPS.append(op)
    dve_ops.CUSTOM_DVE_SPECS[name] = spec
    dve_ops._SUB_OPCODE_FOR_NAME[name] = row
    return op


EXP_OP = _register_exp_op()

# which groups' exp runs on the DVE custom op (the rest on ACT).  2-of-5
# keeps DVE below the PE's per-group budget alongside its normalize work.
_DVE_PAT = os.environ.get("KERNEL_DVE_EXP", "13")


def build_nc():
    nc = bacc.Bacc(target_bir_lowering=False)

    xT_d = nc.dram_tensor("xT", [C, BT], MMDT, kind="ExternalInput")
    w_d = nc.dram_tensor("w", [C, 3 * HPC * D], MMDT, kind="ExternalInput")
    bqk_d = nc.dram_tensor("bqk", [P, 2], F32, kind="ExternalInput")
    wp_d = nc.dram_tensor("wp", [P, C], MMDT, kind="ExternalInput")
    id_d = nc.dram_tensor("ident", [P, P], F32, kind="ExternalInput")
    out_d = nc.dram_tensor("out", [BT, C], BF16, kind="ExternalOutput")

    xT3 = xT_d.ap().rearrange("(ko p) m -> p ko m", p=P)   # [128, 8, 4096]
    w3 = w_d.ap().rearrange("(ko p) n -> p ko n", p=P)     # [128, 8, 384]
    out2 = out_d.ap()                                       # [4096, 1024]

    Exp = mybir.ActivationFunctionType.Exp
    Copy = mybir.ActivationFunctionType.Copy
    mult = mybir.AluOpType.mult
    add = mybir.AluOpType.add

    with tile.TileContext(nc) as tc, ExitStack() as ctx:
        const = ctx.enter_context(tc.tile_pool(name="const", bufs=1))
        big = ctx.enter_context(tc.tile_pool(name="big", bufs=1))
        xpool = ctx.enter_context(tc.tile_pool(name="xpool", bufs=8))
        epool = ctx.enter_context(tc.tile_pool(name="epool", bufs=6))
        spool = ctx.enter_context(tc.tile_pool(name="spool", bufs=2))
        opool = ctx.enter_context(tc.tile_pool(name="opool", bufs=8))
        ypool = ctx.enter_context(tc.tile_pool(name="ypool", bufs=8))
        dpool = ctx.enter_context(tc.tile_pool(name="dpool", bufs=2, space="DRAM"))
        psum = ctx.enter_context(tc.tile_pool(name="psum", bufs=1, space="PSUM"))

        # ---------------- input DMAs ----------------
        # sync queue: x g0 per-k-chunk (interleaved with w on the scalar
        # queue, so the k-th QKV matmul starts when its own slices land).
        # vector queue: wp/ident/bqk consts.  remaining x m-tiles are
        # prefetched whole on rotating engine queues.
        w_sb = const.tile([P, KO, 3 * HPC * D], MMDT)
        wp_sb = const.tile([P, C], MMDT)
        id_sb = const.tile([P, P], F32)
        bqk_sb0 = const.tile([P, 2], F32)
        nc.gpsimd.dma_start(out=wp_sb[:, :], in_=wp_d.ap()[:, :])
        nc.gpsimd.dma_start(out=id_sb[:], in_=id_d.ap())
        nc.gpsimd.dma_start(out=bqk_sb0[:], in_=bqk_d.ap())

        xms = []
        for g in range(B * NMT_B):
            xms.append(xpool.tile([P, KO, MT], MMDT, tag="xm", name=f"xm{g}"))
        for k in range(KO):
            nc.scalar.dma_start(out=w_sb[:, k, :], in_=w3[:, k, :])
            nc.sync.dma_start(out=xms[0][:, k, :], in_=xT3[:, k, 0:MT])
        qs = [nc.sync, nc.gpsimd, nc.scalar]
        for g in range(1, B * NMT_B):
            qs[g % 3].dma_start(out=xms[g][:, :, :],
                                in_=xT3[:, :, g * MT:(g + 1) * MT])

        # Pre-consume DMA semaphores on the engines that will read these
        # tiles (single-wait-slot encodings can't wait (engine, DMA)).
        bqk_q = const.tile([P, 2], F32)     # read by vector (q/k bias)
        nc.vector.tensor_copy(out=bqk_q[:], in_=bqk_sb0[:])
        idb_sb = const.tile([P, P], MMDT)   # read by PE transposes
        nc.scalar.copy(out=idb_sb[:], in_=id_sb[:])

        QT_sb = big.tile([P, B, T], MMDT)   # rows: [qA feats | qB feats]
        KT_sb = big.tile([P, B, T], MMDT)
        VT_sb = big.tile([P, B, T], MMDT)
        # merged V tile: cols 0:64 vA, 64 onesA | 65.. : VB block
        # (65+32 onesB, 65+64..65+128 vB)
        VAB = big.tile([P, B, NKB, 65 + P], MMDT)
        nc.gpsimd.memset(VAB[:], 0.0)
        nc.vector.memset(VAB[:, :, :, 64:65], 1.0)
        nc.vector.memset(VAB[:, :, :, 65 + 32:65 + 33], 1.0)

        # PE warm-up + pre-consume of wp/id DMA sems (results unused).
        pid = psum.tile([P, P], F32, tag="py", bufs=4)
        nc.tensor.transpose(pid[:], id_sb[:], id_sb[:])
        pwp = psum.tile([P, QTW], F32, tag="py", bufs=4)
        nc.tensor.matmul(pwp[:, 0:P], wp_sb[:, 0:P], wp_sb[:, 0:P],
                         start=True, stop=True)
        nc.tensor.matmul(pwp[:, 0:P], wp_sb[:, QTW:QTW + P],
                         wp_sb[:, QTW:QTW + P], start=True, stop=True)

        yts = {}
        pys = {}
        evac_rr = [0]

        def emit_qkv(b):
            for mt in range(NMT_B):
                g = b * NMT_B + mt
                xm = xms[g]
                for nch in range(3):
                    pq = psum.tile([P, MT], F32, tag="py", bufs=4)
                    for k in range(KO):
                        nc.tensor.matmul(
                            pq[:],
                            (w_sb[:, k, nch * P:(nch + 1) * P]),
                            (xm[:, k, :]),
                            start=(k == 0),
                            stop=(k == KO - 1),
                        )
                    dst = (QT_sb, KT_sb, VT_sb)[nch]
                    dslice = dst[:, b, mt * MT:(mt + 1) * MT]
                    if nch == 0:
                        nc.vector.tensor_scalar_add(
                            out=dslice, in0=pq[:], scalar1=bqk_q[:, 0:1])
                    elif nch == 1:
                        nc.vector.tensor_scalar_add(
                            out=dslice, in0=pq[:], scalar1=bqk_q[:, 1:2])
                    else:
                        nc.scalar.copy(out=dslice, in_=pq[:])

        def emit_vtrans(b):
            # V back-transpose to [token, feat]; single strided evac copy
            # per block into the merged VAB tile (A cols 0:64, B 65+64:).
            for kb in range(NKB):
                pt = psum.tile([P, P], MMDT, tag="py", bufs=4)
                nc.tensor.transpose(
                    pt[:], VT_sb[:, b, kb * P:(kb + 1) * P], idb_sb[:])
                dst = VAB[:, b, kb, :]
                dstv = bass.AP(tensor=dst.tensor, offset=dst.offset,
                               ap=[list(dst.ap[0]), [65 + 64, 2], [1, 64]])
                eng = (nc.vector, nc.scalar)[evac_rr[0] % 2]
                evac_rr[0] += 1
                src = pt.rearrange("p (two f) -> p two f", two=2)
                if eng is nc.scalar:
                    eng.copy(out=dstv, in_=src)
                else:
                    eng.tensor_copy(out=dstv, in_=src)

        use_pb = os.environ.get("KERNEL_PB", "0") == "1"

        def emit_normalize(b, qt):
            pyA, pyB = pys[(b, qt)]
            if not use_pb:
                # fallback: baseline-style DRAM bounce broadcast
                yu = spool.tile([P, 2 * QTW], F32, tag="yu", bufs=4,
                                name=f"yu_{b}_{qt}")
                nc.vector.tensor_copy(out=yu[0:65, 0:QTW], in_=pyA[0:65, :])
                nc.vector.tensor_copy(out=yu[0:128, QTW:2 * QTW], in_=pyB[:, :])
                dr = dpool.tile([2, QTW], F32, tag="dr", bufs=3, name=f"dr_{b}_{qt}")
                nc.sync.dma_start(out=dr[1:2, :], in_=yu[64:65, 0:QTW])
                nc.sync.dma_start(out=dr[0:1, :], in_=yu[32:33, QTW:2 * QTW])
                dbx = spool.tile([P, QTW], F32, tag="db", bufs=2, name=f"db_{b}_{qt}")
                rowB, rowA = dr[0:1, :], dr[1:2, :]
                srcA = bass.AP(tensor=rowA.tensor, offset=rowA.offset,
                               ap=[[0, 64], [1, QTW]])
                srcB = bass.AP(tensor=rowB.tensor, offset=rowB.offset,
                               ap=[[0, 64], [1, QTW]])
                nc.sync.dma_start(out=dbx[0:64, :], in_=srcA)
                nc.sync.dma_start(out=dbx[64:128, :], in_=srcB)
                rb = spool.tile([P, QTW], F32, tag="rb", bufs=3, name=f"rb_{b}_{qt}")
                nc.vector.reciprocal_approx_fast(out=rb[:, :], in_=dbx[:, :])
                yTq = ypool.tile([P, QTW], MMDT, tag="yT", name=f"yT_{b}_{qt}")
                yts[(b, qt)] = yTq
                nc.vector.tensor_tensor(
                    yTq[0:64, :], yu[0:64, 0:QTW], rb[0:64, :], mult)
                nc.vector.tensor_tensor(
                    yTq[64:128, :], yu[64:128, QTW:2 * QTW], rb[64:128, :], mult)
                return
            # denominator rows -> 2-partition stage, reciprocal there,
            # broadcast each head's row across its 64 partitions on gpsimd.
            dsA = spool.tile([1, QTW], F32, tag="dsA", bufs=2, name=f"dsA_{b}_{qt}")
            dsB = spool.tile([1, QTW], F32, tag="dsB", bufs=2, name=f"dsB_{b}_{qt}")
            nc.vector.tensor_copy(out=dsA[:, :], in_=pyA[64:65, :])
            nc.vector.tensor_copy(out=dsB[:, :], in_=pyB[32:33, :])
            rsA = spool.tile([1, QTW], F32, tag="rsA", bufs=2, name=f"rsA_{b}_{qt}")
            rsB = spool.tile([1, QTW], F32, tag="rsB", bufs=2, name=f"rsB_{b}_{qt}")
            nc.vector.reciprocal_approx_fast(out=rsA[:, :], in_=dsA[:, :])
            nc.vector.reciprocal_approx_fast(out=rsB[:, :], in_=dsB[:, :])
            db = spool.tile([P, QTW], F32, tag="db", bufs=2, name=f"db_{b}_{qt}")
            nc.gpsimd.partition_broadcast(db[0:64, :], rsA[0:1, :])
            nc.gpsimd.partition_broadcast(db[64:128, :], rsB[0:1, :])
            yTq = ypool.tile([P, QTW], MMDT, tag="yT", name=f"yT_{b}_{qt}")
            yts[(b, qt)] = yTq
            nc.vector.tensor_tensor(
                yTq[0:64, :], pyA[0:64, :], db[0:64, :], mult)
            nc.vector.tensor_tensor(
                yTq[64:128, :], pyB[64:128, :], db[64:128, :], mult)

        def emit_attnv(b, item):
            qt, kb, e, qoff, first, last = item
            if first:
                pyA = psum.tile([P, QTW], F32, tag="py", bufs=4,
                                name=f"pyA_{b}_{qt}")
                pyB = psum.tile([P, QTW], F32, tag="py", bufs=4,
                                name=f"pyB_{b}_{qt}")
                pys[(b, qt)] = (pyA, pyB)
            pyA, pyB = pys[(b, qt)]
            nc.tensor.matmul(
                pyA[0:65, qoff:QTW], (VAB[:, b, kb, 0:65]),
                (e[:, qoff:QTW]),
                start=first, stop=last, skip_group_check=True,
            )
            nc.tensor.matmul(
                pyB[:, qoff:QTW], (VAB[:, b, kb, 65:65 + P]),
                (e[:, QTW + qoff:2 * QTW]),
                start=first, stop=last, skip_group_check=True,
            )
            if last:
                emit_normalize(b, qt)

        def emit_proj(b, qts):
            # out[tok, :] partials; po rides the "ps" ring (idle when no
            # scores are in flight); evacs round-robin across engines.
            for qt in qts:
                yTq = yts[(b, qt)]
                for sm4 in range(4):
                    sm = qt * 4 + sm4
                    po = psum.tile([P, 2, QTW], F32, tag="ps", bufs=2,
                                   name=f"po_{b}_{sm}")
                    osb = opool.tile([P, C], BF16, tag="osb")
                    for nh in range(2):
                        nc.tensor.matmul(
                            po[:, nh, :],
                            (yTq[:, sm4 * P:(sm4 + 1) * P]),
                            (wp_sb[:, nh * QTW:(nh + 1) * QTW]),
                            start=True, stop=True,
                        )
                        eng = (nc.scalar, nc.vector)[evac_rr[0] % 2]
                        evac_rr[0] += 1
                        if eng is nc.scalar:
                            eng.copy(out=osb[:, nh * QTW:(nh + 1) * QTW],
                                     in_=po[:, nh, :])
                        else:
                            eng.tensor_copy(out=osb[:, nh * QTW:(nh + 1) * QTW],
                                            in_=po[:, nh, :])
                    r0 = b * T + sm * P
                    nc.sync.dma_start(out=out2[r0:r0 + P, :], in_=osb[:])

        def emit_attention(b):
            # One flat stream of score-groups, diagonal blocks first within
            # each qt; attnV trails scores by SKEW groups across qt
            # boundaries so the PE never drains waiting on exp.
            SKEW = 4
            groups = []
            for qt in range(NQT):
                order = list(range(qt * 4, (qt + 1) * 4)) + list(range(0, qt * 4))
                for i, kb in enumerate(order):
                    groups.append((qt, kb, i == 0, i == len(order) - 1))

            pend = []
            for gi, (qt, kb, first, last) in enumerate(groups):
                d = kb - (qt * (QTW // P))
                qoff = 0 if os.environ.get("KERNEL_NARROW", "1") == "0" \
                    else max(0, d) * P
                w = QTW - qoff
                q0 = qt * QTW + qoff
                ps = psum.tile([P, 2 * QTW], F32, tag="ps", bufs=2,
                               name=f"ps_{b}_{qt}_{kb}")
                nc.tensor.matmul(
                    ps[:, qoff:QTW],
                    (KT_sb[0:64, b, kb * P:(kb + 1) * P]),
                    (QT_sb[0:64, b, q0:q0 + w]),
                    start=True, stop=True, tile_position=(0, 0),
                )
                nc.tensor.matmul(
                    ps[:, QTW + qoff:2 * QTW],
                    (KT_sb[64:128, b, kb * P:(kb + 1) * P]),
                    (QT_sb[64:128, b, q0:q0 + w]),
                    start=True, stop=True, tile_position=(64, 0),
                )
                e = epool.tile([P, 2 * QTW], MMDT, tag="e", bufs=6,
                               name=f"e_{b}_{qt}_{kb}")
                ps3 = ps.rearrange("p (h q) -> p h q", h=2)
                e3 = e.rearrange("p (h q) -> p h q", h=2)
                use_dve = (str(gi % 5) in _DVE_PAT) and EXP_OP is not None
                if use_dve:
                    nc.vector._custom_dve(
                        EXP_OP, out=e3[:, :, qoff:], in0=ps3[:, :, qoff:],
                        s0=_EXP_C0, s1=_EXP_C1)
                else:
                    nc.scalar.activation(out=e3[:, :, qoff:],
                                         in_=ps3[:, :, qoff:],
                                         func=Exp, scale=SCALE)
                if d >= 0:
                    # within the window: keep exp[j, h, i'] where i' >= j
                    nc.gpsimd.affine_select(
                        out=e3[:, :, qoff:],
                        in_=e3[:, :, qoff:],
                        pattern=[[0, 2], [1, w]],
                        compare_op=mybir.AluOpType.is_ge,
                        fill=0.0,
                        base=0,
                        channel_multiplier=-1,
                    )
                pend.append((qt, kb, e, qoff, first, last))
                if len(pend) > SKEW:
                    emit_attnv(b, pend.pop(0))
            for item in pend:
                emit_attnv(b, item)

        # ---------------- schedule ----------------
        emit_qkv(0)
        emit_vtrans(0)
        emit_attention(0)
        emit_qkv(1)           # PE covers b0's last normalize latency
        emit_vtrans(1)
        emit_proj(0, range(NQT))
        emit_attention(1)
        emit_proj(1, range(NQT - 1))   # qt3's normalize drains under these
        emit_proj(1, [NQT - 1])

    nc.finalize()
    return nc


def prep_inputs(x, W_qkv, b_qkv, W_proj, b_proj):
    """Host-side sharding: returns list of 8 per-core input dicts."""
    import ml_dtypes
    mmnp = ml_dtypes.bfloat16
    x = np.asarray(x, dtype=np.float32)
    W_qkv = np.asarray(W_qkv, dtype=np.float32)
    b_qkv = np.asarray(b_qkv, dtype=np.float32)
    W_proj = np.asarray(W_proj, dtype=np.float32)

    xT = np.ascontiguousarray(x.reshape(BT, C).T).astype(mmnp)   # [C, BT]
    ident = np.eye(P, dtype=np.float32)

    in_maps = []
    for c in range(NCORES):
        hA, hB = HPC * c, HPC * c + 1
        cols = []
        for part in range(3):                               # q, k, v
            for h in (hA, hB):
                cols.append(W_qkv[:, part * C + h * D: part * C + (h + 1) * D])
        w = np.ascontiguousarray(np.concatenate(cols, axis=1)).astype(mmnp)

        bq = np.concatenate([b_qkv[hA * D:(hA + 1) * D], b_qkv[hB * D:(hB + 1) * D]])
        bk = np.concatenate([b_qkv[C + hA * D: C + (hA + 1) * D],
                             b_qkv[C + hB * D: C + (hB + 1) * D]])
        bqk = np.ascontiguousarray(np.stack([bq, bk], axis=1))  # [128, 2]

        wp = np.ascontiguousarray(W_proj[c * P:(c + 1) * P, :]).astype(mmnp)

        in_maps.append({
            "xT": xT,
            "w": w,
            "bqk": bqk,
            "wp": wp,
            "ident": ident,
        })
    return in_maps


_NC_CACHE = None


def kernel(x, W_qkv, b_qkv, W_proj, b_proj):
    global _NC_CACHE, LAST_RESULT
    from concourse.bass_utils import run_bass_kernel_spmd

    if _NC_CACHE is None:
        _NC_CACHE = build_nc()
    nc = _NC_CACHE

    in_maps = prep_inputs(x, W_qkv, b_qkv, W_proj, b_proj)
    trace = os.environ.get("KERNEL_TRACE", "0") == "1"
    res = run_bass_kernel_spmd(nc, in_maps, list(range(NCORES)), trace=trace)
    LAST_RESULT = res

    acc = np.zeros((BT, C), dtype=np.float64)
    for r in res.results:
        acc += r["out"].astype(np.float64)
    # attn rows sum to 1, so the V bias contributes b_v @ W_proj to every
    # token row; add it and the proj bias here (exact, part of unshard).
    W_proj = np.asarray(W_proj, dtype=np.float32)
    b_qkv = np.asarray(b_qkv, dtype=np.float32)
    b_proj = np.asarray(b_proj, dtype=np.float32)
    acc += (b_qkv[2 * C:].astype(np.float64) @ W_proj.astype(np.float64)
            + b_proj.astype(np.float64))
    return acc.astype(np.float32).reshape(B, T, C)
